# revision 26
# baseline (speedup 1.0000x reference)
"""Trainium2 Bass kernel for ContractLevelAttention (segment softmax-pooling).

Computes, for x:[N,D], sorted batch:[N] (graph ids in [0,B)), MLP weights:
    scores = tanh(x @ W1 + b1) @ W2 + b2              # [N]
    w      = segment_softmax(scores, batch)           # per-graph softmax
    out    = segment_sum(x * w[:, None], batch)       # [B, D]

Key facts exploited:
  * softmax is shift invariant and |scores| <= 1 + 128*max|W2| + |b2| ~ 11.5
    (tanh output bounded), so exp() never overflows in fp32 and the
    segment-max subtraction of the reference can be dropped entirely.
  * out[g] = (sum_i e_i x_i) / (sum_i e_i) over i in graph g, so the
    normalization happens once at the end -- both sums are plain
    segment-sums, done as one-hot matmuls on the PE.
  * everything runs in bf16 on the PE (1 cycle/row vs 4 for fp32) with
    fp32 PSUM accumulation; x is shipped to HBM as bf16, halving DMA.
  * exp is computed as e^s = (1+t)/(1-t) with t = tanh((s+b2)/2): the Act
    engine has no table set containing both Tanh and Exp, so using Exp
    would force a 1.3us act-table reload twice per chunk. Tanh+Copy share
    a set, so the whole kernel runs on one act table.
  * emission is software-pipelined: pooling of chunk k-1 is emitted after
    the score pass of chunk k, and within the score pass transposes run
    one supertile ahead of the MLP matmul (m2 lags two) so the PE never
    waits on Act/DVE. One-hot generation alternates DVE/GPSIMD.

Sharding: graph-level data parallel over 8 cores (batch is sorted, so each
core's nodes are one contiguous slice, zero-padded to a fixed capacity).
"""

import numpy as np
import ml_dtypes
from contextlib import ExitStack

N_FULL = 524288
D = 256
H = 128
B_FULL = 2048
NCORES = 8
B_LOC = B_FULL // NCORES      # 256 graphs per core
GCH = 128                     # graphs per PSUM accumulator chunk
PAD_SENTINEL = 3.0 * B_LOC    # brel value for padding rows (never matches)
CHT = 24                      # 128-node tiles per x DMA chunk (24*128
                              # divides the 67584-node capacity exactly)
STT = 4                       # tiles per compute supertile

BF16 = ml_dtypes.bfloat16

_prog_cache = {}


def _build_program(C, t_lo1, t_hi0, repeat=1, ablate=""):
    """Build the per-core SPMD program. C = padded node capacity (multiple of
    128*CHT). Chunk 0 (graphs 0..127 of this core) covers node tiles
    [0, t_hi0); chunk 1 (graphs 128..255) covers [t_lo1, T). repeat>1 wraps
    the whole body in an on-device loop (for timing)."""
    import concourse.bass as bass
    from concourse import bacc, mybir
    import concourse.tile as tile

    f32 = mybir.dt.float32
    bf16 = mybir.dt.bfloat16
    AFT = mybir.ActivationFunctionType
    ALU = mybir.AluOpType
    T = C // 128
    NST = CHT // STT  # supertiles per chunk

    nc = bacc.Bacc(
        "TRN2",
        target_bir_lowering=False,
        debug=False,
        enable_asserts=False,
        num_devices=NCORES,
    )
    x_d = nc.dram_tensor("x", [C, D], bf16, kind="ExternalInput").ap()
    brel_d = nc.dram_tensor("brel", [128, T], f32, kind="ExternalInput").ap()
    w1_d = nc.dram_tensor("w1", [2, 128, H], bf16, kind="ExternalInput").ap()
    b1_d = nc.dram_tensor("b1", [H, 1], f32, kind="ExternalInput").ap()
    w2_d = nc.dram_tensor("w2", [H, 1], bf16, kind="ExternalInput").ap()
    b2h_d = nc.dram_tensor("b2h", [128, 1], f32, kind="ExternalInput").ap()
    id_d = nc.dram_tensor("ident", [128, 128], bf16, kind="ExternalInput").ap()
    iota_d = nc.dram_tensor("iota", [128, B_LOC], bf16, kind="ExternalInput").ap()
    out_d = nc.dram_tensor("out", [B_LOC, D], f32, kind="ExternalOutput").ap()

    first = {0: 0, 1: t_lo1}
    last = {0: t_hi0 - 1, 1: T - 1}

    with tile.TileContext(nc) as tc, ExitStack() as ctx:
        const = ctx.enter_context(tc.tile_pool(name="const", bufs=1))
        xp = ctx.enter_context(tc.tile_pool(name="xp", bufs=4))
        xtp = ctx.enter_context(tc.tile_pool(name="xtp", bufs=4))
        ttp = ctx.enter_context(tc.tile_pool(name="ttp", bufs=5))
        ep = ctx.enter_context(tc.tile_pool(name="ep", bufs=2))
        thp = ctx.enter_context(tc.tile_pool(name="thp", bufs=2))
        oep = ctx.enter_context(tc.tile_pool(name="oep", bufs=6))
        outp = ctx.enter_context(tc.tile_pool(name="outp", bufs=2))
        smallp = ctx.enter_context(tc.tile_pool(name="smallp", bufs=6))
        ps_xt = ctx.enter_context(tc.tile_pool(name="ps_xt", bufs=3, space="PSUM"))
        ps_u = ctx.enter_context(tc.tile_pool(name="ps_u", bufs=2, space="PSUM"))
        ps_s = ctx.enter_context(tc.tile_pool(name="ps_s", bufs=1, space="PSUM"))
        ps_acc = ctx.enter_context(tc.tile_pool(name="ps_acc", bufs=2, space="PSUM"))

        # --- constants, loaded once ---
        w1_s = const.tile([128, 256], bf16)
        nc.sync.dma_start(w1_s[:, 0:128], w1_d[0])
        nc.sync.dma_start(w1_s[:, 128:256], w1_d[1])
        brel_s = const.tile([128, T], f32)
        nc.sync.dma_start(brel_s[:], brel_d[:])
        b1_s = const.tile([128, 1], f32)
        nc.sync.dma_start(b1_s[:], b1_d[:])
        w2_s = const.tile([128, 1], bf16)
        nc.sync.dma_start(w2_s[:], w2_d[:])
        b2h_s = const.tile([128, 1], f32)
        nc.sync.dma_start(b2h_s[:], b2h_d[:])
        id_s = const.tile([128, 128], bf16)
        nc.sync.dma_start(id_s[:], id_d[:])
        iota_s = const.tile([128, B_LOC], bf16)
        nc.sync.dma_start(iota_s[:], iota_d[:])
        ones_s = const.tile([128, 1], bf16)
        nc.vector.memset(ones_s[:], 1.0)

        score_on = ablate not in ("noscore", "dmaonly")
        trans_on = score_on and ablate != "notrans"
        pool_on = ablate not in ("nopool", "dmaonly")
        if not trans_on:
            xdum_s = const.tile([128, 2 * STT * 128], bf16)
            nc.vector.memset(xdum_s[:], 0.01)

        def emit_transposes(xc, t0, si):
            """Transpose supertile si of the chunk at t0 into one PSUM bank,
            then copy both halves (Act + DVE in parallel) into SBUF bf16."""
            if not trans_on:
                return xdum_s
            st = si * STT
            xt_s = xtp.tile([128, 2 * STT * 128], bf16, tag="xts")
            xt_sv = xt_s[:, :].rearrange("p (c q) -> p c q", c=2)
            xt_ps = ps_xt.tile([128, 1024], bf16, tag="xtps")
            for h in (0, 1):
                for jj in (0, 1):
                    j = st + 2 * h + jj
                    for c in (0, 1):
                        nc.tensor.transpose(
                            xt_ps[:, h * 512 + c * 256 + jj * 128 :
                                  h * 512 + c * 256 + jj * 128 + 128],
                            xc[:, j * D + c * 128 : j * D + c * 128 + 128],
                            id_s[:],
                        )
            for h in (0, 1):
                src = xt_ps[:, h * 512 : h * 512 + 512].rearrange(
                    "p (c r) -> p c r", c=2
                )
                dst = xt_sv[:, :, h * 256 : h * 256 + 256]
                if h == 0:
                    nc.scalar.copy(dst, src)
                else:
                    nc.vector.tensor_copy(dst, src)
            return xt_s

        def emit_m1(xt_s, si):
            """MLP layer 1 + tanh for supertile si -> tt tile [128, 512]."""
            u_ps = ps_u.tile([128, 512], f32, tag="ups")
            nc.tensor.matmul(
                u_ps[:], w1_s[:, 0:128], xt_s[:, 0:512], start=True, stop=False
            )
            nc.tensor.matmul(
                u_ps[:], w1_s[:, 128:256], xt_s[:, 512:1024],
                start=False, stop=True,
            )
            tt_s = ttp.tile([128, 512], bf16, tag="tts")
            nc.scalar.activation(tt_s[:], u_ps[:], AFT.Tanh, bias=b1_s[:])
            return tt_s

        def emit_m2(tt_s, s_ps, si):
            for j in range(STT):
                col = si * STT + j
                nc.tensor.matmul(
                    s_ps[:, col : col + 1],
                    tt_s[:, j * 128 : (j + 1) * 128],
                    w2_s[:],
                    start=True,
                    stop=True,
                )

        def emit_score(xc, t0):
            """Score pass for the chunk at t0 -> e_s [128, CHT] (f32)."""
            e_s = ep.tile([128, CHT], f32, tag="es")
            if not score_on:
                nc.vector.memset(e_s[:], 1.0)
                return e_s
            s_ps = ps_s.tile([128, CHT], f32, tag="sps")
            # software-pipelined supertile loop: T(si) || m1(si-1) || m2(si-3)
            xts = {}
            tts = {}
            for ph in range(NST + 5):
                if ph < NST:
                    xts[ph] = emit_transposes(xc, t0, ph)
                if 2 <= ph < NST + 2:
                    tts[ph - 2] = emit_m1(xts.pop(ph - 2), ph - 2)
                if ph >= 5:
                    emit_m2(tts.pop(ph - 5), s_ps, ph - 5)
            # e = (1+t)/(1-t), t = tanh(s/2 + b2/2)  (exact identity for e^s)
            th_s = thp.tile([128, CHT], f32, tag="ths")
            nc.scalar.activation(th_s[:], s_ps[:], AFT.Tanh, bias=b2h_s[:], scale=0.5)
            num = smallp.tile([128, CHT], f32, tag="num")
            nc.vector.tensor_scalar_add(num[:], th_s[:], 1.0)
            dnm = smallp.tile([128, CHT], f32, tag="dnm")
            nc.vector.tensor_scalar(
                dnm[:], th_s[:], -1.0, 1.0, op0=ALU.mult, op1=ALU.add
            )
            rcp = smallp.tile([128, CHT], f32, tag="rcp")
            nc.vector.reciprocal(rcp[:], dnm[:])
            nc.vector.tensor_tensor(e_s[:], num[:], rcp[:], op=ALU.mult)
            return e_s

        def emit_pool(xc, e_s, t0, acc):
            for j in range(CHT):
                t = t0 + j
                x_t = xc[:, j * D : (j + 1) * D]
                for c in (0, 1):
                    if not (first[c] <= t <= last[c]):
                        continue
                    if t == first[c]:
                        acc_t = ps_acc.tile([128, 257], f32, tag="acc")
                        acc[c] = acc_t
                        # den col accumulates with start=False throughout
                        # (a second start=True matmul on this bank would
                        # wipe the data region) -- zero it explicitly
                        nc.vector.memset(acc_t[:, D : D + 1], 0.0)
                    oe_s = oep.tile([128, GCH], bf16, tag="oes")
                    eng = nc.vector
                    eng.tensor_scalar(
                        oe_s[:],
                        iota_s[:, c * GCH : (c + 1) * GCH],
                        brel_s[:, t : t + 1],
                        e_s[:, j : j + 1],
                        op0=ALU.is_equal,
                        op1=ALU.mult,
                    )
                    nc.tensor.matmul(
                        acc[c][:, 0:D],
                        oe_s[:],
                        x_t[:],
                        start=(t == first[c]),
                        stop=(t == last[c]),
                    )
                    nc.tensor.matmul(
                        acc[c][:, D : D + 1],
                        oe_s[:],
                        ones_s[:],
                        start=False,
                        stop=(t == last[c]),
                        skip_group_check=True,
                    )
                    if t == last[c]:
                        den = smallp.tile([128, 1], f32, tag="den")
                        nc.vector.tensor_scalar_add(
                            den[:], acc[c][:, D : D + 1], 1e-30
                        )
                        rec = smallp.tile([128, 1], f32, tag="rec")
                        nc.vector.reciprocal(rec[:], den[:])
                        o_s = outp.tile([128, D], f32, tag="os")
                        nc.vector.tensor_scalar_mul(o_s[:], acc[c][:, 0:D], rec[:])
                        nc.sync.dma_start(out_d[c * GCH : (c + 1) * GCH, :], o_s[:])

        def body(_iv=None):
            acc = {}
            pend = None  # (xc, e_s, t0) of the chunk whose pooling is deferred
            for t0 in range(0, T, CHT):
                xc = xp.tile([128, CHT * D], bf16, tag="xc")
                nc.sync.dma_start(
                    xc[:, :].rearrange("p (j d) -> p j d", d=D),
                    x_d[t0 * 128 : (t0 + CHT) * 128, :].rearrange(
                        "(j p) d -> p j d", p=128
                    ),
                )
                e_s = emit_score(xc, t0)
                if pool_on and pend is not None:
                    emit_pool(*pend, acc)
                pend = (xc, e_s, t0)
            if pool_on and pend is not None:
                emit_pool(*pend, acc)

        if repeat == 1:
            body()
        else:
            with tc.For_i(0, repeat, 1) as _i:
                body(_i)
    nc.compile()
    return nc


def _get_program(C, t_lo1, t_hi0, repeat=1, ablate=""):
    key = (C, t_lo1, t_hi0, repeat, ablate)
    if key not in _prog_cache:
        _prog_cache[key] = _build_program(C, t_lo1, t_hi0, repeat, ablate)
    return _prog_cache[key]


def _prep_inputs(x, batch, W1, b1, W2, b2):
    """Host-side sharding: split nodes at graph boundaries, pad to fixed C.
    x / weights are cast to bf16 host-side (halves HBM traffic; PE runs
    bf16 at 4x the fp32 rate)."""
    x = np.ascontiguousarray(x, dtype=np.float32)
    batch = np.asarray(batch)
    W1 = np.ascontiguousarray(W1, dtype=np.float32)

    bounds = np.searchsorted(batch, np.arange(0, B_FULL + 1, B_LOC))  # [9]
    mids = np.searchsorted(batch, np.arange(GCH, B_FULL, B_LOC))  # chunk mids [8]
    n_k = np.diff(bounds)
    cap = int(n_k.max())
    gran = 128 * CHT
    C = max(67584, ((cap + gran - 1) // gran) * gran)
    C = ((C + gran - 1) // gran) * gran
    T = C // 128

    b_rel = mids - bounds[:-1]
    t_lo1 = int(min(b_rel // 128))
    t_hi0 = int(max((b_rel - 1) // 128) + 1)
    t_lo1 = max(0, min(t_lo1, T))
    t_hi0 = max(1, min(t_hi0, T))

    shared = {
        "w1": W1.reshape(2, 128, H).astype(BF16),
        "b1": np.ascontiguousarray(b1, dtype=np.float32).reshape(H, 1),
        "w2": np.ascontiguousarray(W2, dtype=np.float32).reshape(H, 1).astype(BF16),
        "b2h": np.full(
            (128, 1), 0.5 * float(np.asarray(b2).reshape(-1)[0]), np.float32
        ),
        "ident": np.eye(128, dtype=BF16),
        "iota": np.broadcast_to(
            np.arange(B_LOC, dtype=BF16), (128, B_LOC)
        ).copy(),
    }
    in_maps = []
    for k in range(NCORES):
        s, e = int(bounds[k]), int(bounds[k + 1])
        n = e - s
        xk = np.zeros((C, D), BF16)
        xk[:n] = x[s:e]
        br = np.full((C,), PAD_SENTINEL, np.float32)
        br[:n] = batch[s:e].astype(np.float32) - k * B_LOC
        in_maps.append(
            {
                "x": xk,
                "brel": np.ascontiguousarray(br.reshape(T, 128).T),
                **shared,
            }
        )
    return in_maps, C, t_lo1, t_hi0


def kernel(x, batch, W1, b1, W2, b2):
    from concourse.bass_utils import run_bass_kernel_spmd

    in_maps, C, t_lo1, t_hi0 = _prep_inputs(x, batch, W1, b1, W2, b2)
    nc = _get_program(C, t_lo1, t_hi0)
    res = run_bass_kernel_spmd(nc, in_maps, list(range(NCORES)))
    out = np.concatenate([res.results[k]["out"] for k in range(NCORES)], axis=0)
    return np.ascontiguousarray(out, dtype=np.float32)


# revision 29
# speedup vs baseline: 1.0852x; 1.0852x over previous
"""Trainium2 Bass kernel for ContractLevelAttention (segment softmax-pooling).

Computes, for x:[N,D], sorted batch:[N] (graph ids in [0,B)), MLP weights:
    scores = tanh(x @ W1 + b1) @ W2 + b2              # [N]
    w      = segment_softmax(scores, batch)           # per-graph softmax
    out    = segment_sum(x * w[:, None], batch)       # [B, D]

Key facts exploited:
  * softmax is shift invariant and |scores| <= 1 + 128*max|W2| + |b2| ~ 11.5
    (tanh output bounded), so exp() never overflows in fp32 and the
    segment-max subtraction of the reference can be dropped entirely.
  * out[g] = (sum_i e_i x_i) / (sum_i e_i) over i in graph g, so the
    normalization happens once at the end -- both sums are plain
    segment-sums, done as one-hot matmuls on the PE.
  * everything runs in bf16 on the PE (1 cycle/row vs 4 for fp32) with
    fp32 PSUM accumulation; x is shipped to HBM as bf16, halving DMA.
  * exp is computed as e^s = (1+t)/(1-t) with t = tanh((s+b2)/2): the Act
    engine has no table set containing both Tanh and Exp, so using Exp
    would force a 1.3us act-table reload twice per chunk. Tanh+Copy share
    a set, so the whole kernel runs on one act table.
  * emission is software-pipelined: pooling of chunk k-1 is emitted after
    the score pass of chunk k, and within the score pass transposes run
    one supertile ahead of the MLP matmul (m2 lags two) so the PE never
    waits on Act/DVE. One-hot generation alternates DVE/GPSIMD.

Sharding: graph-level data parallel over 8 cores (batch is sorted, so each
core's nodes are one contiguous slice, zero-padded to a fixed capacity).
"""

import numpy as np
import ml_dtypes
from contextlib import ExitStack

N_FULL = 524288
D = 256
H = 128
B_FULL = 2048
NCORES = 8
B_LOC = B_FULL // NCORES      # 256 graphs per core
GCH = 128                     # graphs per PSUM accumulator chunk
PAD_SENTINEL = 3.0 * B_LOC    # brel value for padding rows (never matches)
CHT = 24                      # 128-node tiles per x DMA chunk (24*128
                              # divides the 67584-node capacity exactly)
STT = 4                       # tiles per compute supertile

BF16 = ml_dtypes.bfloat16

_prog_cache = {}


def _build_program(C, t_lo1, t_hi0, repeat=1, ablate=""):
    """Build the per-core SPMD program. C = padded node capacity (multiple of
    128*CHT). Chunk 0 (graphs 0..127 of this core) covers node tiles
    [0, t_hi0); chunk 1 (graphs 128..255) covers [t_lo1, T). repeat>1 wraps
    the whole body in an on-device loop (for timing)."""
    import concourse.bass as bass
    from concourse import bacc, mybir
    import concourse.tile as tile

    f32 = mybir.dt.float32
    bf16 = mybir.dt.bfloat16
    AFT = mybir.ActivationFunctionType
    ALU = mybir.AluOpType
    T = C // 128
    NST = CHT // STT  # supertiles per chunk

    nc = bacc.Bacc(
        "TRN2",
        target_bir_lowering=False,
        debug=False,
        enable_asserts=False,
        num_devices=NCORES,
    )
    x_d = nc.dram_tensor("x", [C, D], bf16, kind="ExternalInput").ap()
    brel_d = nc.dram_tensor("brel", [128, T], f32, kind="ExternalInput").ap()
    w1_d = nc.dram_tensor("w1", [2, 128, H], bf16, kind="ExternalInput").ap()
    b1_d = nc.dram_tensor("b1", [H, 1], f32, kind="ExternalInput").ap()
    w2_d = nc.dram_tensor("w2", [H, 1], bf16, kind="ExternalInput").ap()
    b2h_d = nc.dram_tensor("b2h", [128, 1], f32, kind="ExternalInput").ap()
    id_d = nc.dram_tensor("ident", [128, 128], bf16, kind="ExternalInput").ap()
    iota_d = nc.dram_tensor("iota", [128, B_LOC], bf16, kind="ExternalInput").ap()
    out_d = nc.dram_tensor("out", [B_LOC, D], f32, kind="ExternalOutput").ap()

    first = {0: 0, 1: t_lo1}
    last = {0: t_hi0 - 1, 1: T - 1}

    with tile.TileContext(nc) as tc, ExitStack() as ctx:
        const = ctx.enter_context(tc.tile_pool(name="const", bufs=1))
        xp = ctx.enter_context(tc.tile_pool(name="xp", bufs=4))
        xtp = ctx.enter_context(tc.tile_pool(name="xtp", bufs=4))
        ttp = ctx.enter_context(tc.tile_pool(name="ttp", bufs=5))
        ep = ctx.enter_context(tc.tile_pool(name="ep", bufs=2))
        thp = ctx.enter_context(tc.tile_pool(name="thp", bufs=2))
        oep = ctx.enter_context(tc.tile_pool(name="oep", bufs=6))
        outp = ctx.enter_context(tc.tile_pool(name="outp", bufs=2))
        smallp = ctx.enter_context(tc.tile_pool(name="smallp", bufs=6))
        ps_xt = ctx.enter_context(tc.tile_pool(name="ps_xt", bufs=3, space="PSUM"))
        ps_u = ctx.enter_context(tc.tile_pool(name="ps_u", bufs=2, space="PSUM"))
        ps_s = ctx.enter_context(tc.tile_pool(name="ps_s", bufs=1, space="PSUM"))
        ps_acc = ctx.enter_context(tc.tile_pool(name="ps_acc", bufs=2, space="PSUM"))

        # --- constants, loaded once ---
        w1_s = const.tile([128, 256], bf16)
        nc.sync.dma_start(w1_s[:, 0:128], w1_d[0])
        nc.sync.dma_start(w1_s[:, 128:256], w1_d[1])
        brel_s = const.tile([128, T], f32)
        nc.sync.dma_start(brel_s[:], brel_d[:])
        b1_s = const.tile([128, 1], f32)
        nc.sync.dma_start(b1_s[:], b1_d[:])
        w2_s = const.tile([128, 1], bf16)
        nc.sync.dma_start(w2_s[:], w2_d[:])
        b2h_s = const.tile([128, 1], f32)
        nc.sync.dma_start(b2h_s[:], b2h_d[:])
        id_s = const.tile([128, 128], bf16)
        nc.sync.dma_start(id_s[:], id_d[:])
        iota_s = const.tile([128, B_LOC], bf16)
        nc.sync.dma_start(iota_s[:], iota_d[:])
        ones_s = const.tile([128, 1], bf16)
        nc.vector.memset(ones_s[:], 1.0)

        score_on = ablate not in ("noscore", "dmaonly")
        trans_on = score_on and ablate != "notrans"
        pool_on = ablate not in ("nopool", "dmaonly")
        if not trans_on:
            xdum_s = const.tile([128, 2 * STT * 128], bf16)
            nc.vector.memset(xdum_s[:], 0.01)

        def emit_transposes(xc, t0, si):
            """Transpose supertile si of the chunk at t0 into one PSUM bank,
            then copy both halves (Act + DVE in parallel) into SBUF bf16."""
            if not trans_on:
                return xdum_s
            st = si * STT
            xt_s = xtp.tile([128, 2 * STT * 128], bf16, tag="xts")
            xt_sv = xt_s[:, :].rearrange("p (c q) -> p c q", c=2)
            xt_ps = ps_xt.tile([128, 1024], bf16, tag="xtps")
            for h in (0, 1):
                for jj in (0, 1):
                    j = st + 2 * h + jj
                    for c in (0, 1):
                        nc.tensor.transpose(
                            xt_ps[:, h * 512 + c * 256 + jj * 128 :
                                  h * 512 + c * 256 + jj * 128 + 128],
                            xc[:, j * D + c * 128 : j * D + c * 128 + 128],
                            id_s[:],
                        )
            for h in (0, 1):
                src = xt_ps[:, h * 512 : h * 512 + 512].rearrange(
                    "p (c r) -> p c r", c=2
                )
                dst = xt_sv[:, :, h * 256 : h * 256 + 256]
                if h == 0:
                    nc.scalar.copy(dst, src)
                else:
                    nc.vector.tensor_copy(dst, src)
            return xt_s

        def emit_m1(xt_s, si):
            """MLP layer 1 + tanh for supertile si -> tt tile [128, 512]."""
            u_ps = ps_u.tile([128, 512], f32, tag="ups")
            nc.tensor.matmul(
                u_ps[:], w1_s[:, 0:128], xt_s[:, 0:512], start=True, stop=False
            )
            nc.tensor.matmul(
                u_ps[:], w1_s[:, 128:256], xt_s[:, 512:1024],
                start=False, stop=True,
            )
            tt_s = ttp.tile([128, 512], bf16, tag="tts")
            nc.scalar.activation(tt_s[:], u_ps[:], AFT.Tanh, bias=b1_s[:])
            return tt_s

        def emit_m2(tt_s, s_ps, si):
            for j in range(STT):
                col = si * STT + j
                nc.tensor.matmul(
                    s_ps[:, col : col + 1],
                    tt_s[:, j * 128 : (j + 1) * 128],
                    w2_s[:],
                    start=True,
                    stop=True,
                )

        def emit_score(xc, t0):
            """Score pass for the chunk at t0 -> e_s [128, CHT] (f32)."""
            e_s = ep.tile([128, CHT], f32, tag="es")
            if not score_on:
                nc.vector.memset(e_s[:], 1.0)
                return e_s
            s_ps = ps_s.tile([128, CHT], f32, tag="sps")
            # software-pipelined supertile loop: T(si) || m1(si-1) || m2(si-3)
            xts = {}
            tts = {}
            for ph in range(NST + 5):
                if ph < NST:
                    xts[ph] = emit_transposes(xc, t0, ph)
                if 2 <= ph < NST + 2:
                    tts[ph - 2] = emit_m1(xts.pop(ph - 2), ph - 2)
                if ph >= 5:
                    emit_m2(tts.pop(ph - 5), s_ps, ph - 5)
            # e = (1+t)/(1-t), t = tanh(s/2 + b2/2)  (exact identity for e^s)
            th_s = thp.tile([128, CHT], f32, tag="ths")
            nc.scalar.activation(th_s[:], s_ps[:], AFT.Tanh, bias=b2h_s[:], scale=0.5)
            num = smallp.tile([128, CHT], f32, tag="num")
            nc.vector.tensor_scalar_add(num[:], th_s[:], 1.0)
            dnm = smallp.tile([128, CHT], f32, tag="dnm")
            nc.vector.tensor_scalar(
                dnm[:], th_s[:], -1.0, 1.0, op0=ALU.mult, op1=ALU.add
            )
            rcp = smallp.tile([128, CHT], f32, tag="rcp")
            nc.vector.reciprocal(rcp[:], dnm[:])
            nc.vector.tensor_tensor(e_s[:], num[:], rcp[:], op=ALU.mult)
            return e_s

        def emit_pool(xc, e_s, t0, acc):
            for j in range(CHT):
                t = t0 + j
                x_t = xc[:, j * D : (j + 1) * D]
                for c in (0, 1):
                    if not (first[c] <= t <= last[c]):
                        continue
                    if t == first[c]:
                        acc_t = ps_acc.tile([128, 257], f32, tag="acc")
                        acc[c] = acc_t
                        # den col accumulates with start=False throughout
                        # (a second start=True matmul on this bank would
                        # wipe the data region) -- zero it explicitly
                        nc.vector.memset(acc_t[:, D : D + 1], 0.0)
                    oe_s = oep.tile([128, GCH], bf16, tag="oes")
                    eng = nc.vector
                    eng.tensor_scalar(
                        oe_s[:],
                        iota_s[:, c * GCH : (c + 1) * GCH],
                        brel_s[:, t : t + 1],
                        e_s[:, j : j + 1],
                        op0=ALU.is_equal,
                        op1=ALU.mult,
                    )
                    nc.tensor.matmul(
                        acc[c][:, 0:D],
                        oe_s[:],
                        x_t[:],
                        start=(t == first[c]),
                        stop=(t == last[c]),
                    )
                    nc.tensor.matmul(
                        acc[c][:, D : D + 1],
                        oe_s[:],
                        ones_s[:],
                        start=False,
                        stop=(t == last[c]),
                        skip_group_check=True,
                    )
                    if t == last[c]:
                        den = smallp.tile([128, 1], f32, tag="den")
                        nc.vector.tensor_scalar_add(
                            den[:], acc[c][:, D : D + 1], 1e-30
                        )
                        rec = smallp.tile([128, 1], f32, tag="rec")
                        nc.vector.reciprocal(rec[:], den[:])
                        o_s = outp.tile([128, D], f32, tag="os")
                        nc.vector.tensor_scalar_mul(o_s[:], acc[c][:, 0:D], rec[:])
                        nc.sync.dma_start(out_d[c * GCH : (c + 1) * GCH, :], o_s[:])

        def body(_iv=None):
            acc = {}
            pend = None  # (xc, e_s, t0) of the chunk whose pooling is deferred
            for t0 in range(0, T, CHT):
                xc = xp.tile([128, CHT * D], bf16, tag="xc")
                xc_v = xc[:, :].rearrange("p (j d) -> p j d", d=D)
                # split the chunk load across 4 DMAs -> parallel DMA queues
                qt = CHT // 4
                for q in range(4):
                    nc.sync.dma_start(
                        xc_v[:, q * qt : (q + 1) * qt, :],
                        x_d[(t0 + q * qt) * 128 : (t0 + (q + 1) * qt) * 128, :]
                        .rearrange("(j p) d -> p j d", p=128),
                    )
                e_s = emit_score(xc, t0)
                if pool_on and pend is not None:
                    emit_pool(*pend, acc)
                pend = (xc, e_s, t0)
            if pool_on and pend is not None:
                emit_pool(*pend, acc)

        if repeat == 1:
            body()
        else:
            with tc.For_i(0, repeat, 1) as _i:
                body(_i)
    nc.compile()
    return nc


def _get_program(C, t_lo1, t_hi0, repeat=1, ablate=""):
    key = (C, t_lo1, t_hi0, repeat, ablate)
    if key not in _prog_cache:
        _prog_cache[key] = _build_program(C, t_lo1, t_hi0, repeat, ablate)
    return _prog_cache[key]


def _prep_inputs(x, batch, W1, b1, W2, b2):
    """Host-side sharding: split nodes at graph boundaries, pad to fixed C.
    x / weights are cast to bf16 host-side (halves HBM traffic; PE runs
    bf16 at 4x the fp32 rate)."""
    x = np.ascontiguousarray(x, dtype=np.float32)
    batch = np.asarray(batch)
    W1 = np.ascontiguousarray(W1, dtype=np.float32)

    bounds = np.searchsorted(batch, np.arange(0, B_FULL + 1, B_LOC))  # [9]
    mids = np.searchsorted(batch, np.arange(GCH, B_FULL, B_LOC))  # chunk mids [8]
    n_k = np.diff(bounds)
    cap = int(n_k.max())
    gran = 128 * CHT
    C = max(67584, ((cap + gran - 1) // gran) * gran)
    C = ((C + gran - 1) // gran) * gran
    T = C // 128

    b_rel = mids - bounds[:-1]
    t_lo1 = int(min(b_rel // 128))
    t_hi0 = int(max((b_rel - 1) // 128) + 1)
    t_lo1 = max(0, min(t_lo1, T))
    t_hi0 = max(1, min(t_hi0, T))

    shared = {
        "w1": W1.reshape(2, 128, H).astype(BF16),
        "b1": np.ascontiguousarray(b1, dtype=np.float32).reshape(H, 1),
        "w2": np.ascontiguousarray(W2, dtype=np.float32).reshape(H, 1).astype(BF16),
        "b2h": np.full(
            (128, 1), 0.5 * float(np.asarray(b2).reshape(-1)[0]), np.float32
        ),
        "ident": np.eye(128, dtype=BF16),
        "iota": np.broadcast_to(
            np.arange(B_LOC, dtype=BF16), (128, B_LOC)
        ).copy(),
    }
    in_maps = []
    for k in range(NCORES):
        s, e = int(bounds[k]), int(bounds[k + 1])
        n = e - s
        xk = np.zeros((C, D), BF16)
        xk[:n] = x[s:e]
        br = np.full((C,), PAD_SENTINEL, np.float32)
        br[:n] = batch[s:e].astype(np.float32) - k * B_LOC
        in_maps.append(
            {
                "x": xk,
                "brel": np.ascontiguousarray(br.reshape(T, 128).T),
                **shared,
            }
        )
    return in_maps, C, t_lo1, t_hi0


def kernel(x, batch, W1, b1, W2, b2):
    from concourse.bass_utils import run_bass_kernel_spmd

    in_maps, C, t_lo1, t_hi0 = _prep_inputs(x, batch, W1, b1, W2, b2)
    nc = _get_program(C, t_lo1, t_hi0)
    res = run_bass_kernel_spmd(nc, in_maps, list(range(NCORES)))
    out = np.concatenate([res.results[k]["out"] for k in range(NCORES)], axis=0)
    return np.ascontiguousarray(out, dtype=np.float32)
